# revision 33
# baseline (speedup 1.0000x reference)
"""Trainium2 Bass kernel for nn_Classification_4922032521468.

Problem: acts = embeds[activity_index]  (A=512 rows, d=512)
         pairs = concat(acts[ii], acts[jj])  for all i<j (P=130816 pairs)
         out = log_softmax(pairs @ W.T + b)  -> [P, 4]

Key algebra: logits[p, c] = L[i, c] + R'[j, c]  with
  L  = acts @ Wl.T          (Wl = W[:, :512])
  R' = acts @ Wr.T + b      (Wr = W[:, 512:])
so log_softmax needs only lse[i, j] = ln(sum_c e^{L[i,c]} e^{R'[j,c]})
and  out[i, j, c] = L[i, c] + R'[j, c] - lse[i, j].
No 130816x1024 pair tensor is ever built.

Sharding: 2D tile - core k = (a = k%4, b2 = k//4) owns the
[128 i x 256 j] tile of the 512x512 (i, j) square.

Work split: the host does the O(input)-sized preprocessing - the row
gather, the [A, C] projections L/R' (4 output columns), their exps,
and the operand layouts below. The device does ALL O(P) output-scale
compute: the pairwise lse matmuls, the Ln, the pair-plane broadcast
matmuls, the log-softmax combine, and the full [P, 4] output
materialization + store. (Shipping raw acts instead is 784KB/core of
input DMA - measured as the dominant critical path; the projections
compress that to 15KB.)

Per-core inputs (two DMAs on the two HWDGE queues):
  uv [4, 384] fp16 (SP queue): [ut = e^{L^T} (128) | vt = e^{(R'+b)^T}]
  auxf [8, 768] fp16 (ACT queue):
    cols 0:512 (combo): rows 0:4 = cones (c'==c blocks),
                        rows 4:8 = ltm[c',128c+i] = L^T[c',i]*(c'==c)
    cols 512:768 (lhs): rows 0:4 = rt = (R'+b)^T, rows 4:8 = 1.0
  (host-built, so no engine ever writes them - DMA writes have no
  partition-alignment constraint and the K=8 reads start at 0)

Device graph per core (4 matmuls, 2 ACT ops, 3 DVE ops, 2+3 DMAs):
  se3[j, 128jc+i] = vt_jc^T @ ut        2 matmuls (K=4)
  lnse_jc = Ln(se3_jc)                  2 ACT [128,128]
  pre_jc[j, 128c+i] = lhs_jc^T @ combo  1 matmul/jc (K=8, PSUM)
                      = L[i,c] + R'[j,c] + b[c]
  osb = pre - lnse (broadcast over c)   DVE fp16 (jc0 whole, jc1 halves)
  stores: jc0 [128,512]; jc1 split into column halves across SP/ACT.

num_devices=1 (no collectives). Host reassembles the 8 [256, 512]
tiles into out_sq[i, j, c] and extracts the triu pairs.
"""

import numpy as np

A = 512  # number of activity tokens
D = 512  # embedding dim
C = 4  # classes
IB = 128  # i-rows per core
JB = 256  # j-cols per core
NCORES = 8

_program = None
_last_results = None  # BassKernelResults from the most recent run (profiling)


def _build_program():
    from contextlib import ExitStack

    import concourse.bacc as bacc
    import concourse.mybir as mybir
    import concourse.tile as tile
    from concourse.tile_rust import add_dep_helper

    fp32 = mybir.dt.float32
    fp16 = mybir.dt.float16
    AF = mybir.ActivationFunctionType
    SUB = mybir.AluOpType.subtract

    nc = bacc.Bacc(
        "TRN2",
        target_bir_lowering=False,
        debug=False,
        enable_asserts=False,
        num_devices=1,
    )

    # fold operands [8, 768]: cols 0:512 = combo (rows 0:4 cones, rows
    # 4:8 ltm), cols 512:768 = lhs (rows 0:4 rt, rows 4:8 ones). All
    # host-built, so no engine ever writes them and the K=8 stack needs
    # no partition-alignment padding.
    auxf_h = nc.dram_tensor("auxf", (8, 768), fp16, kind="ExternalInput")
    # uv [4, 384]: [ut = e^{L^T} (128) | vt = e^{(R'+b)^T} (256)]
    uv_h = nc.dram_tensor("uv", (4, 384), fp16, kind="ExternalInput")
    # out[j, 128c + i]
    out_h = nc.dram_tensor("out", (JB, IB * C), fp16, kind="ExternalOutput")
    out_ap = out_h.ap()

    with tile.TileContext(nc) as tc, ExitStack() as ctx:
        sb = ctx.enter_context(tc.tile_pool(name="sb", bufs=1))
        sbr = ctx.enter_context(tc.tile_pool(name="sbr", bufs=2))
        psS = ctx.enter_context(tc.tile_pool(name="psS", bufs=1, space="PSUM"))
        psB = ctx.enter_context(tc.tile_pool(name="psB", bufs=2, space="PSUM"))

        # tiny uv lands first on the SP queue and unblocks the lse matmuls;
        # the fold operands ride the ACT queue in parallel
        uv = sb.tile([4, 384], fp16, tag="uv")
        nc.sync.dma_start(out=uv[:], in_=uv_h.ap()[:])

        # manual Ln-covering ACT table load, emitted before the ACT-queue
        # DMA: it starts as soon as the auto-inserted top load finishes
        # (table loads serialize on the table-DMA path while the queue
        # keeps dispatching), and it keeps the auto-insertion pass from
        # adding a third load between the DMA and the Ln
        ldtab = nc.scalar.add_instruction(
            mybir.InstLoadActFuncSet(
                act_func_set_id=6,  # natural_log_exp_and_others
                name=f"I-{nc.next_id()}",
                engine=mybir.EngineType.Activation,
            )
        )
        aux = sb.tile([8, 768], fp16, tag="aux")
        nc.scalar.dma_start(out=aux[:], in_=auxf_h.ap()[:])

        combo = aux[:, 0:512]
        lhs = aux[:, 512:768]
        ut = uv[:, 0:128]
        vt = uv[:, 128:384]

        # ---- lse: se3[j, 128jc+i] = sum_c V[c,j] U[c,i]; lnse = Ln ----
        se3 = psS.tile([128, 2 * IB], fp32, tag="se3")
        for jc in range(2):
            nc.tensor.matmul(
                out=se3[:, IB * jc : IB * (jc + 1)],
                lhsT=vt[:, IB * jc : IB * (jc + 1)],
                rhs=ut[:],
                start=True,
                stop=True,
            )
        lnse = sb.tile([128, 2 * IB], fp32, tag="lnse")
        for jc in range(2):
            ln_i = nc.scalar.activation(
                out=lnse[:, IB * jc : IB * (jc + 1)],
                in_=se3[:, IB * jc : IB * (jc + 1)],
                func=AF.Ln,
            )
            add_dep_helper(ln_i.ins, ldtab.ins, sync=False, reason="act-table")

        # ---- per jc: pre = lhs_jc^T @ combo (K=36); osb = pre - lnse ----
        for jc in range(2):
            pre = psB.tile([128, IB * C], fp32, tag="pre", name="pre")
            nc.tensor.matmul(
                out=pre[:],
                lhsT=lhs[:, IB * jc : IB * (jc + 1)],
                rhs=combo[:],
                start=True,
                stop=True,
            )
            osb = sbr.tile([128, IB * C], fp16, tag="osb", name="osb")
            if jc == 0:
                nc.vector.tensor_tensor(
                    out=osb[:].rearrange("p (i c) -> p i c", c=C),
                    in0=pre[:].rearrange("p (i c) -> p i c", c=C),
                    in1=lnse[:, 0:IB].unsqueeze(2).to_broadcast([128, IB, C]),
                    op=SUB,
                )
                nc.sync.dma_start(out=out_ap[0:128, :], in_=osb[:])
            else:
                # tail combine + store split into column halves across the
                # SP and ACT HWDGE queues: each half stores as soon as its
                # DVE combine finishes
                for h in range(2):
                    cs = 256 * h
                    nc.vector.tensor_tensor(
                        out=osb[:, cs : cs + 256].rearrange(
                            "p (i c) -> p i c", c=C
                        ),
                        in0=pre[:, cs : cs + 256].rearrange(
                            "p (i c) -> p i c", c=C
                        ),
                        in1=lnse[:, IB + 64 * h : IB + 64 * (h + 1)]
                        .unsqueeze(2)
                        .to_broadcast([128, 64, C]),
                        op=SUB,
                    )
                    eng = nc.sync if h == 0 else nc.scalar
                    eng.dma_start(
                        out=out_ap[128:256, cs : cs + 256],
                        in_=osb[:, cs : cs + 256],
                    )

    nc.compile()
    return nc


def _get_program():
    global _program
    if _program is None:
        _program = _build_program()
    return _program


def kernel(embeds, activity_index, W, b):
    from concourse.bass_utils import run_bass_kernel_spmd

    embeds = np.asarray(embeds, dtype=np.float32)
    W = np.asarray(W, dtype=np.float32)
    b_in = np.asarray(b, dtype=np.float32).reshape(C)
    idx = np.asarray(activity_index).astype(np.int64)

    # host-side O(input) preprocessing: gather + the [A, C] projections
    acts = embeds[idx]  # [512, 512]
    L = acts @ W[:, :D].T  # [512, 4]
    R = acts @ W[:, D:].T + b_in  # [512, 4] (bias folded)
    eL = np.exp(L)
    eR = np.exp(R)

    in_maps = []
    for k in range(NCORES):
        a, b2 = k % 4, k // 4
        isl = slice(IB * a, IB * (a + 1))
        jsl = slice(JB * b2, JB * (b2 + 1))
        auxf = np.zeros((8, 768), dtype=np.float16)
        ic = np.arange(IB) * C
        for c in range(C):
            auxf[c, ic + c] = 1.0  # cones (i-major, c-minor plane)
            auxf[4 + c, ic + c] = L[isl, c]  # ltm
        auxf[0:4, 512:768] = R[jsl].T  # rt rows of the fold stationary
        auxf[4:8, 512:768] = 1.0  # ones rows
        uv = np.empty((4, 384), dtype=np.float16)
        uv[:, 0:128] = eL[isl].T  # ut
        uv[:, 128:384] = eR[jsl].T  # vt
        in_maps.append(
            {"auxf": np.ascontiguousarray(auxf), "uv": np.ascontiguousarray(uv)}
        )

    nc = _get_program()
    results = run_bass_kernel_spmd(nc, in_maps, core_ids=list(range(NCORES)))
    global _last_results
    _last_results = results

    out_sq = np.empty((A, A, C), dtype=np.float32)
    for k in range(NCORES):
        a, b2 = k % 4, k // 4
        # blk[j_loc, i_loc, c] -> out_sq[i, j, c]
        blk = results.results[k]["out"].reshape(JB, IB, C).astype(np.float32)
        out_sq[IB * a : IB * (a + 1), JB * b2 : JB * (b2 + 1), :] = blk.transpose(
            1, 0, 2
        )

    ii, jj = np.triu_indices(A, k=1)
    return np.ascontiguousarray(out_sq[ii, jj])


# revision 34
# speedup vs baseline: 1.0091x; 1.0091x over previous
"""Trainium2 Bass kernel for nn_Classification_4922032521468.

Problem: acts = embeds[activity_index]  (A=512 rows, d=512)
         pairs = concat(acts[ii], acts[jj])  for all i<j (P=130816 pairs)
         out = log_softmax(pairs @ W.T + b)  -> [P, 4]

Key algebra: logits[p, c] = L[i, c] + R'[j, c]  with
  L  = acts @ Wl.T          (Wl = W[:, :512])
  R' = acts @ Wr.T + b      (Wr = W[:, 512:])
so log_softmax needs only lse[i, j] = ln(sum_c e^{L[i,c]} e^{R'[j,c]})
and  out[i, j, c] = L[i, c] + R'[j, c] - lse[i, j].
No 130816x1024 pair tensor is ever built.

Sharding: 2D tile - core k = (a = k%4, b2 = k//4) owns the
[128 i x 256 j] tile of the 512x512 (i, j) square.

Work split: the host does the O(input)-sized preprocessing - the row
gather, the [A, C] projections L/R' (4 output columns), their exps,
and the operand layouts below. The device does ALL O(P) output-scale
compute: the pairwise lse matmuls, the Ln, the pair-plane broadcast
matmuls, the log-softmax combine, and the full [P, 4] output
materialization + store. (Shipping raw acts instead is 784KB/core of
input DMA - measured as the dominant critical path; the projections
compress that to 15KB.)

Per-core inputs (two DMAs on the two HWDGE queues):
  uv [4, 384] fp16 (SP queue): [ut = e^{L^T} (128) | vt = e^{(R'+b)^T}]
  auxf [8, 768] fp16 (ACT queue):
    cols 0:512 (combo): rows 0:4 = cones (c'==c blocks),
                        rows 4:8 = ltm[c',128c+i] = L^T[c',i]*(c'==c)
    cols 512:768 (lhs): rows 0:4 = rt = (R'+b)^T, rows 4:8 = 1.0
  (host-built, so no engine ever writes them - DMA writes have no
  partition-alignment constraint and the K=8 reads start at 0)

Device graph per core (4 matmuls, 2 ACT ops, 3 DVE ops, 2+3 DMAs):
  se3[j, 128jc+i] = vt_jc^T @ ut        2 matmuls (K=4)
  lnse_jc = Ln(se3_jc)                  2 ACT [128,128]
  pre_jc[j, 128c+i] = lhs_jc^T @ combo  1 matmul/jc (K=8, PSUM)
                      = L[i,c] + R'[j,c] + b[c]
  osb = pre - lnse (broadcast over c)   DVE fp16 (jc0 whole, jc1 halves)
  stores: jc0 [128,512]; jc1 split into column halves across SP/ACT.

num_devices=1 (no collectives). Host reassembles the 8 [256, 512]
tiles into out_sq[i, j, c] and extracts the triu pairs.
"""

import numpy as np

A = 512  # number of activity tokens
D = 512  # embedding dim
C = 4  # classes
IB = 128  # i-rows per core
JB = 256  # j-cols per core
NCORES = 8

_program = None
_last_results = None  # BassKernelResults from the most recent run (profiling)


def _build_program():
    from contextlib import ExitStack

    import concourse.bacc as bacc
    import concourse.mybir as mybir
    import concourse.tile as tile
    from concourse.tile_rust import add_dep_helper

    fp32 = mybir.dt.float32
    fp16 = mybir.dt.float16
    AF = mybir.ActivationFunctionType
    SUB = mybir.AluOpType.subtract

    nc = bacc.Bacc(
        "TRN2",
        target_bir_lowering=False,
        debug=False,
        enable_asserts=False,
        num_devices=1,
    )

    # fold operands [8, 768]: cols 0:512 = combo (rows 0:4 cones, rows
    # 4:8 ltm), cols 512:768 = lhs (rows 0:4 rt, rows 4:8 ones). All
    # host-built, so no engine ever writes them and the K=8 stack needs
    # no partition-alignment padding.
    auxf_h = nc.dram_tensor("auxf", (8, 768), fp16, kind="ExternalInput")
    # uv [4, 384]: [ut = e^{L^T} (128) | vt = e^{(R'+b)^T} (256)]
    uv_h = nc.dram_tensor("uv", (4, 384), fp16, kind="ExternalInput")
    # out[j, 128c + i]
    out_h = nc.dram_tensor("out", (JB, IB * C), fp16, kind="ExternalOutput")
    out_ap = out_h.ap()

    with tile.TileContext(nc) as tc, ExitStack() as ctx:
        sb = ctx.enter_context(tc.tile_pool(name="sb", bufs=1))
        sbr = ctx.enter_context(tc.tile_pool(name="sbr", bufs=2))
        psS = ctx.enter_context(tc.tile_pool(name="psS", bufs=1, space="PSUM"))
        psB = ctx.enter_context(tc.tile_pool(name="psB", bufs=2, space="PSUM"))

        # tiny uv lands first on the SP queue and unblocks the lse matmuls;
        # the fold operands ride the ACT queue in parallel
        uv = sb.tile([4, 384], fp16, tag="uv")
        nc.sync.dma_start(out=uv[:], in_=uv_h.ap()[:])

        # manual Ln-covering ACT table load, emitted before the ACT-queue
        # DMA: it starts as soon as the auto-inserted top load finishes
        # (table loads serialize on the table-DMA path while the queue
        # keeps dispatching), and it keeps the auto-insertion pass from
        # adding a third load between the DMA and the Ln
        ldtab = nc.scalar.add_instruction(
            mybir.InstLoadActFuncSet(
                act_func_set_id=6,  # natural_log_exp_and_others
                name=f"I-{nc.next_id()}",
                engine=mybir.EngineType.Activation,
            )
        )
        aux = sb.tile([8, 768], fp16, tag="aux")
        nc.scalar.dma_start(out=aux[:], in_=auxf_h.ap()[:])

        combo = aux[:, 0:512]
        lhs = aux[:, 512:768]
        ut = uv[:, 0:128]
        vt = uv[:, 128:384]

        # ---- lse: se3[j, 128jc+i] = sum_c V[c,j] U[c,i]; lnse = Ln ----
        se3 = psS.tile([128, 2 * IB], fp32, tag="se3")
        for jc in range(2):
            nc.tensor.matmul(
                out=se3[:, IB * jc : IB * (jc + 1)],
                lhsT=vt[:, IB * jc : IB * (jc + 1)],
                rhs=ut[:],
                start=True,
                stop=True,
            )
        lnse = sb.tile([128, 2 * IB], fp32, tag="lnse")
        for jc in range(2):
            ln_i = nc.scalar.activation(
                out=lnse[:, IB * jc : IB * (jc + 1)],
                in_=se3[:, IB * jc : IB * (jc + 1)],
                func=AF.Ln,
            )
            add_dep_helper(ln_i.ins, ldtab.ins, sync=False, reason="act-table")

        # ---- per jc: pre = lhs_jc^T @ combo (K=8); osb = pre - lnse ----
        for jc in range(2):
            pre = psB.tile([128, IB * C], fp32, tag="pre", name="pre")
            nc.tensor.matmul(
                out=pre[:],
                lhsT=lhs[:, IB * jc : IB * (jc + 1)],
                rhs=combo[:],
                start=True,
                stop=True,
            )
            osb = sbr.tile([128, IB * C], fp16, tag="osb", name="osb")
            if jc == 0:
                nc.vector.tensor_tensor(
                    out=osb[:].rearrange("p (c i) -> p c i", c=C),
                    in0=pre[:].rearrange("p (c i) -> p c i", c=C),
                    in1=lnse[:, 0:IB].unsqueeze(1).to_broadcast([128, C, IB]),
                    op=SUB,
                )
                nc.sync.dma_start(out=out_ap[0:128, :], in_=osb[:])
            else:
                # tail combine + store split into column halves across the
                # SP and ACT HWDGE queues: each half stores as soon as its
                # DVE combine finishes
                for h in range(2):
                    cs = 256 * h
                    nc.vector.tensor_tensor(
                        out=osb[:, cs : cs + 256].rearrange(
                            "p (c i) -> p c i", c=2
                        ),
                        in0=pre[:, cs : cs + 256].rearrange(
                            "p (c i) -> p c i", c=2
                        ),
                        in1=lnse[:, IB : 2 * IB]
                        .unsqueeze(1)
                        .to_broadcast([128, 2, IB]),
                        op=SUB,
                    )
                    eng = nc.sync if h == 0 else nc.scalar
                    eng.dma_start(
                        out=out_ap[128:256, cs : cs + 256],
                        in_=osb[:, cs : cs + 256],
                    )

    nc.compile()
    return nc


def _get_program():
    global _program
    if _program is None:
        _program = _build_program()
    return _program


def kernel(embeds, activity_index, W, b):
    from concourse.bass_utils import run_bass_kernel_spmd

    embeds = np.asarray(embeds, dtype=np.float32)
    W = np.asarray(W, dtype=np.float32)
    b_in = np.asarray(b, dtype=np.float32).reshape(C)
    idx = np.asarray(activity_index).astype(np.int64)

    # host-side O(input) preprocessing: gather + the [A, C] projections
    acts = embeds[idx]  # [512, 512]
    L = acts @ W[:, :D].T  # [512, 4]
    R = acts @ W[:, D:].T + b_in  # [512, 4] (bias folded)
    eL = np.exp(L)
    eR = np.exp(R)

    in_maps = []
    for k in range(NCORES):
        a, b2 = k % 4, k // 4
        isl = slice(IB * a, IB * (a + 1))
        jsl = slice(JB * b2, JB * (b2 + 1))
        auxf = np.zeros((8, 768), dtype=np.float16)
        for c in range(C):
            auxf[c, 128 * c : 128 * (c + 1)] = 1.0  # cones
            auxf[4 + c, 128 * c : 128 * (c + 1)] = L[isl, c]  # ltm
        auxf[0:4, 512:768] = R[jsl].T  # rt rows of the fold stationary
        auxf[4:8, 512:768] = 1.0  # ones rows
        uv = np.empty((4, 384), dtype=np.float16)
        uv[:, 0:128] = eL[isl].T  # ut
        uv[:, 128:384] = eR[jsl].T  # vt
        in_maps.append(
            {"auxf": np.ascontiguousarray(auxf), "uv": np.ascontiguousarray(uv)}
        )

    nc = _get_program()
    results = run_bass_kernel_spmd(nc, in_maps, core_ids=list(range(NCORES)))
    global _last_results
    _last_results = results

    out_sq = np.empty((A, A, C), dtype=np.float32)
    for k in range(NCORES):
        a, b2 = k % 4, k // 4
        # blk[j_loc, c, i_loc] -> out_sq[i, j, c]
        blk = results.results[k]["out"].reshape(JB, C, IB).astype(np.float32)
        out_sq[IB * a : IB * (a + 1), JB * b2 : JB * (b2 + 1), :] = blk.transpose(
            2, 0, 1
        )

    ii, jj = np.triu_indices(A, k=1)
    return np.ascontiguousarray(out_sq[ii, jj])


# revision 35
# speedup vs baseline: 1.0196x; 1.0104x over previous
"""Trainium2 Bass kernel for nn_Classification_4922032521468.

Problem: acts = embeds[activity_index]  (A=512 rows, d=512)
         pairs = concat(acts[ii], acts[jj])  for all i<j (P=130816 pairs)
         out = log_softmax(pairs @ W.T + b)  -> [P, 4]

Key algebra: logits[p, c] = L[i, c] + R'[j, c]  with
  L  = acts @ Wl.T          (Wl = W[:, :512])
  R' = acts @ Wr.T + b      (Wr = W[:, 512:])
so log_softmax needs only lse[i, j] = ln(sum_c e^{L[i,c]} e^{R'[j,c]})
and  out[i, j, c] = L[i, c] + R'[j, c] - lse[i, j].
No 130816x1024 pair tensor is ever built.

Sharding: 2D tile - core k = (a = k%4, b2 = k//4) owns the
[128 i x 256 j] tile of the 512x512 (i, j) square.

Work split: the host does the O(input)-sized preprocessing - the row
gather, the [A, C] projections L/R' (4 output columns), their exps,
and the operand layouts below. The device does ALL O(P) output-scale
compute: the pairwise lse matmuls, the Ln, the pair-plane broadcast
matmuls, the log-softmax combine, and the full [P, 4] output
materialization + store. (Shipping raw acts instead is 784KB/core of
input DMA - measured as the dominant critical path; the projections
compress that to 15KB.)

Per-core inputs (two DMAs on the two HWDGE queues):
  uv [4, 384] fp16 (SP queue): [ut = e^{L^T} (128) | vt = e^{(R'+b)^T}]
  auxf [8, 768] fp16 (ACT queue):
    cols 0:512 (combo): rows 0:4 = cones (c'==c blocks),
                        rows 4:8 = ltm[c',128c+i] = L^T[c',i]*(c'==c)
    cols 512:768 (lhs): rows 0:4 = rt = (R'+b)^T, rows 4:8 = 1.0
  (host-built, so no engine ever writes them - DMA writes have no
  partition-alignment constraint and the K=8 reads start at 0)

Device graph per core (4 matmuls, 2 ACT ops, 3 DVE ops, 2+3 DMAs):
  se3[j, 128jc+i] = vt_jc^T @ ut        2 matmuls (K=4)
  lnse_jc = Ln(se3_jc)                  2 ACT [128,128]
  pre_jc[j, 128c+i] = lhs_jc^T @ combo  1 matmul/jc (K=8, PSUM)
                      = L[i,c] + R'[j,c] + b[c]
  osb = pre - lnse (broadcast over c)   DVE fp16 (jc0 whole, jc1 halves)
  stores: jc0 [128,512]; jc1 split into column halves across SP/ACT.

num_devices=1 (no collectives). Host reassembles the 8 [256, 512]
tiles into out_sq[i, j, c] and extracts the triu pairs.
"""

import numpy as np

A = 512  # number of activity tokens
D = 512  # embedding dim
C = 4  # classes
IB = 128  # i-rows per core
JB = 256  # j-cols per core
NCORES = 8

_program = None
_last_results = None  # BassKernelResults from the most recent run (profiling)


def _build_program():
    from contextlib import ExitStack

    import concourse.bacc as bacc
    import concourse.mybir as mybir
    import concourse.tile as tile
    from concourse.tile_rust import add_dep_helper

    fp32 = mybir.dt.float32
    fp16 = mybir.dt.float16
    AF = mybir.ActivationFunctionType
    SUB = mybir.AluOpType.subtract

    nc = bacc.Bacc(
        "TRN2",
        target_bir_lowering=False,
        debug=False,
        enable_asserts=False,
        num_devices=1,
    )

    # fold operands [8, 768]: cols 0:512 = combo (rows 0:4 cones, rows
    # 4:8 ltm), cols 512:768 = lhs (rows 0:4 rt, rows 4:8 ones). All
    # host-built, so no engine ever writes them and the K=8 stack needs
    # no partition-alignment padding.
    auxf_h = nc.dram_tensor("auxf", (8, 768), fp16, kind="ExternalInput")
    # uv [4, 384]: [ut = e^{L^T} (128) | vt = e^{(R'+b)^T} (256)]
    uv_h = nc.dram_tensor("uv", (4, 384), fp16, kind="ExternalInput")
    # out[j, 128c + i]
    out_h = nc.dram_tensor("out", (JB, IB * C), fp16, kind="ExternalOutput")
    out_ap = out_h.ap()

    with tile.TileContext(nc) as tc, ExitStack() as ctx:
        sb = ctx.enter_context(tc.tile_pool(name="sb", bufs=1))
        sbr = ctx.enter_context(tc.tile_pool(name="sbr", bufs=2))
        psS = ctx.enter_context(tc.tile_pool(name="psS", bufs=1, space="PSUM"))
        psB = ctx.enter_context(tc.tile_pool(name="psB", bufs=2, space="PSUM"))

        # tiny uv lands first on the SP queue and unblocks the lse matmuls;
        # the fold operands ride the ACT queue in parallel
        uv = sb.tile([4, 384], fp16, tag="uv")
        nc.sync.dma_start(out=uv[:], in_=uv_h.ap()[:])

        # manual Ln-covering ACT table load, emitted before the ACT-queue
        # DMA: it starts as soon as the auto-inserted top load finishes
        # (table loads serialize on the table-DMA path while the queue
        # keeps dispatching), and it keeps the auto-insertion pass from
        # adding a third load between the DMA and the Ln
        ldtab = nc.scalar.add_instruction(
            mybir.InstLoadActFuncSet(
                act_func_set_id=6,  # natural_log_exp_and_others
                name=f"I-{nc.next_id()}",
                engine=mybir.EngineType.Activation,
            )
        )
        aux = sb.tile([8, 768], fp16, tag="aux")
        nc.scalar.dma_start(out=aux[:], in_=auxf_h.ap()[:])

        combo = aux[:, 0:512]
        lhs = aux[:, 512:768]
        ut = uv[:, 0:128]
        vt = uv[:, 128:384]

        # ---- lse + fold, PE-interleaved per jc so pre_jc is ready as
        # soon as possible after its lnse_jc:
        #   se3[j, 128jc+i] = sum_c V[c,j] U[c,i]   (K=4)
        #   pre_jc[j, 128c+i] = lhs_jc^T @ combo    (K=8)
        #                     = L[i,c] + R'[j,c] + b[c]
        se3 = psS.tile([128, 2 * IB], fp32, tag="se3")
        pres = []
        for jc in range(2):
            nc.tensor.matmul(
                out=se3[:, IB * jc : IB * (jc + 1)],
                lhsT=vt[:, IB * jc : IB * (jc + 1)],
                rhs=ut[:],
                start=True,
                stop=True,
            )
            pre = psB.tile([128, IB * C], fp32, tag="pre", name="pre")
            nc.tensor.matmul(
                out=pre[:],
                lhsT=lhs[:, IB * jc : IB * (jc + 1)],
                rhs=combo[:],
                start=True,
                stop=True,
            )
            pres.append(pre)
        lnse = sb.tile([128, 2 * IB], fp32, tag="lnse")
        for jc in range(2):
            ln_i = nc.scalar.activation(
                out=lnse[:, IB * jc : IB * (jc + 1)],
                in_=se3[:, IB * jc : IB * (jc + 1)],
                func=AF.Ln,
            )
            add_dep_helper(ln_i.ins, ldtab.ins, sync=False, reason="act-table")

        # ---- per jc: osb = pre - lnse (broadcast over c); store ----
        for jc in range(2):
            pre = pres[jc]
            osb = sbr.tile([128, IB * C], fp16, tag="osb", name="osb")
            if jc == 0:
                nc.vector.tensor_tensor(
                    out=osb[:].rearrange("p (c i) -> p c i", c=C),
                    in0=pre[:].rearrange("p (c i) -> p c i", c=C),
                    in1=lnse[:, 0:IB].unsqueeze(1).to_broadcast([128, C, IB]),
                    op=SUB,
                )
                nc.sync.dma_start(out=out_ap[0:128, :], in_=osb[:])
            else:
                # tail combine + store split into column halves across the
                # SP and ACT HWDGE queues: each half stores as soon as its
                # DVE combine finishes
                for h in range(2):
                    cs = 256 * h
                    nc.vector.tensor_tensor(
                        out=osb[:, cs : cs + 256].rearrange(
                            "p (c i) -> p c i", c=2
                        ),
                        in0=pre[:, cs : cs + 256].rearrange(
                            "p (c i) -> p c i", c=2
                        ),
                        in1=lnse[:, IB : 2 * IB]
                        .unsqueeze(1)
                        .to_broadcast([128, 2, IB]),
                        op=SUB,
                    )
                    eng = nc.sync if h == 0 else nc.scalar
                    eng.dma_start(
                        out=out_ap[128:256, cs : cs + 256],
                        in_=osb[:, cs : cs + 256],
                    )

    nc.compile()
    return nc


def _get_program():
    global _program
    if _program is None:
        _program = _build_program()
    return _program


def kernel(embeds, activity_index, W, b):
    from concourse.bass_utils import run_bass_kernel_spmd

    embeds = np.asarray(embeds, dtype=np.float32)
    W = np.asarray(W, dtype=np.float32)
    b_in = np.asarray(b, dtype=np.float32).reshape(C)
    idx = np.asarray(activity_index).astype(np.int64)

    # host-side O(input) preprocessing: gather + the [A, C] projections
    acts = embeds[idx]  # [512, 512]
    L = acts @ W[:, :D].T  # [512, 4]
    R = acts @ W[:, D:].T + b_in  # [512, 4] (bias folded)
    eL = np.exp(L)
    eR = np.exp(R)

    in_maps = []
    for k in range(NCORES):
        a, b2 = k % 4, k // 4
        isl = slice(IB * a, IB * (a + 1))
        jsl = slice(JB * b2, JB * (b2 + 1))
        auxf = np.zeros((8, 768), dtype=np.float16)
        for c in range(C):
            auxf[c, 128 * c : 128 * (c + 1)] = 1.0  # cones
            auxf[4 + c, 128 * c : 128 * (c + 1)] = L[isl, c]  # ltm
        auxf[0:4, 512:768] = R[jsl].T  # rt rows of the fold stationary
        auxf[4:8, 512:768] = 1.0  # ones rows
        uv = np.empty((4, 384), dtype=np.float16)
        uv[:, 0:128] = eL[isl].T  # ut
        uv[:, 128:384] = eR[jsl].T  # vt
        in_maps.append(
            {"auxf": np.ascontiguousarray(auxf), "uv": np.ascontiguousarray(uv)}
        )

    nc = _get_program()
    results = run_bass_kernel_spmd(nc, in_maps, core_ids=list(range(NCORES)))
    global _last_results
    _last_results = results

    out_sq = np.empty((A, A, C), dtype=np.float32)
    for k in range(NCORES):
        a, b2 = k % 4, k // 4
        # blk[j_loc, c, i_loc] -> out_sq[i, j, c]
        blk = results.results[k]["out"].reshape(JB, C, IB).astype(np.float32)
        out_sq[IB * a : IB * (a + 1), JB * b2 : JB * (b2 + 1), :] = blk.transpose(
            2, 0, 1
        )

    ii, jj = np.triu_indices(A, k=1)
    return np.ascontiguousarray(out_sq[ii, jj])
